# revision 6
# baseline (speedup 1.0000x reference)
"""KNNGraph (k=16) Bass kernel for 8 NeuronCores.

Input: x (4, 8192, 64) fp32. Output: (src, dst) int32 edge arrays of the
16-NN graph per batch (self included), matching jax.lax.top_k(-d2) order.

Sharding: core c handles batch c//2, query rows (c%2)*4096 ... +4096,
against all 8192 keys of that batch (query-row sharding, keys replicated).

Device (per core), per group of 128 query rows x 8 chunks of 1024 keys:
  PE : 2 bf16 matmuls (K=66: 64 dims + hi/lo split of -|key|^2/2)
       -> PSUM (128, 1024) of w = q.k - |k|^2/2 (rank-equiv to -d2/2)
  ACT: convert PSUM fp32 -> bf16 written into the HIGH u16 halves of a
       u32 "packed" tile whose LOW halves hold a constant u16 iota
       (key index within chunk), i.e. packed = bf16(w)<<16 | idx.
  DVE: one max8 over the packed tile viewed as fp32 -> top-8 (value,
       index) pairs per chunk in a single pass (no max_index rescans).
Host: decode 64 candidates/row, exact fp64 re-rank of top-32 by device
value, suspect detection (chunk-8th or 33rd candidate near the 16th
pick) with exact full-row recompute for the rare flagged rows.
"""

import numpy as np

N, M, D = 4, 8192, 64
K = 16
NCORES = 8
QROWS = M // 2           # query rows per core
NGROUPS = QROWS // 128   # 32
CHUNK = 1024             # keys per chunk
NCHUNK = M // CHUNK      # 8
KDIM = D + 2             # contraction rows: 64 dims + hi/lo of -|k|^2/2
NCAND = NCHUNK * 8       # 64 candidates per row
TSEL = 32                # host: exact-rank this many candidates
MARGIN = 1.0             # suspect margin in device w units

_COMPILED = {}
LAST_RESULTS = {}


def _build_nc(reps=1):
    import concourse.bacc as bacc
    import concourse.mybir as mybir
    import concourse.tile as tile

    nc = bacc.Bacc(None)
    f32 = mybir.dt.float32
    bf16 = mybir.dt.bfloat16
    u32 = mybir.dt.uint32

    q_d = nc.declare_dram_parameter("q", [KDIM, QROWS], bf16, isOutput=False)
    kv_d = nc.declare_dram_parameter("kv", [KDIM, M], bf16, isOutput=False)
    pinit_d = nc.declare_dram_parameter("pinit", [128, 2 * CHUNK], u32, isOutput=False)
    c8_d = nc.declare_dram_parameter("c8", [NGROUPS, 128, NCAND], f32, isOutput=True)

    with tile.TileContext(nc) as tc:
        with (
            tc.tile_pool(name="singles", bufs=1) as singles,
            tc.tile_pool(name="psum", bufs=4, space="PSUM") as psum,
            tc.tile_pool(name="cands", bufs=2) as cands,
        ):
            q_sb = singles.tile([KDIM, QROWS], bf16)
            kv_sb = singles.tile([KDIM, M], bf16)
            packed = singles.tile([128, 2 * CHUNK], u32)
            nc.gpsimd.dma_start(out=q_sb[:], in_=q_d[:])
            nc.gpsimd.dma_start(out=kv_sb[:], in_=kv_d[:])
            nc.gpsimd.dma_start(out=packed[:], in_=pinit_d[:])

            for _ in range(reps):
                for g in range(NGROUPS):
                    c8 = cands.tile([128, NCAND], f32, tag="c8")
                    lhsT = q_sb[:, g * 128:(g + 1) * 128]
                    for c in range(NCHUNK):
                        off = (c % 2) * CHUNK
                        pt = psum.tile([128, CHUNK], f32, tag="pt")
                        nc.tensor.matmul(
                            pt[:, 0:512], lhsT,
                            kv_sb[:, c * CHUNK:c * CHUNK + 512],
                            start=True, stop=True,
                        )
                        nc.tensor.matmul(
                            pt[:, 512:CHUNK], lhsT,
                            kv_sb[:, c * CHUNK + 512:(c + 1) * CHUNK],
                            start=True, stop=True,
                        )
                        hi = (
                            packed[:, off:off + CHUNK]
                            .bitcast(bf16)
                            .rearrange("p (n two) -> p n two", two=2)[:, :, 1]
                        )
                        nc.scalar.activation(
                            out=hi, in_=pt[:],
                            func=mybir.ActivationFunctionType.Copy,
                        )
                        nc.vector.max(
                            out=c8[:, c * 8:(c + 1) * 8],
                            in_=packed[:, off:off + CHUNK].bitcast(f32),
                        )
                    nc.sync.dma_start(out=c8_d[g], in_=c8[:])
    if not nc.is_finalized():
        nc.finalize()
    return nc


def _prep_inputs(x):
    """Per-core input dicts. x: (N, M, D) fp32."""
    import ml_dtypes

    bf = ml_dtypes.bfloat16
    x64 = x.astype(np.float64)
    x2 = (x64 * x64).sum(-1)                  # (N, M)
    nh = -0.5 * x2
    nh_hi = nh.astype(bf)
    nh_lo = (nh - nh_hi.astype(np.float64)).astype(bf)
    xb = x.astype(bf)
    ones = np.ones((), bf)

    iota = np.arange(CHUNK, dtype=np.uint32)
    pinit = np.tile(np.concatenate([iota, iota])[None, :], (128, 1)).copy()

    in_maps = []
    for c in range(NCORES):
        b, h = c // 2, c % 2
        q = np.zeros((KDIM, QROWS), bf)
        q[:D] = xb[b, h * QROWS:(h + 1) * QROWS, :].T
        q[D] = ones
        q[D + 1] = ones
        kv = np.zeros((KDIM, M), bf)
        kv[:D] = xb[b].T
        kv[D] = nh_hi[b]
        kv[D + 1] = nh_lo[b]
        in_maps.append({"q": q, "kv": kv, "pinit": pinit})
    return in_maps


def _host_merge(x, bits):
    """bits: (N, M, NCAND) u32 packed candidates. Returns idx (N, M, K) i64."""
    x64 = x.astype(np.float64)
    x2 = (x64 * x64).sum(-1)                              # (N, M)
    loc = (bits & np.uint32(0xFFFF)).astype(np.int64)
    chunk_off = (np.arange(NCAND, dtype=np.int64) // 8) * CHUNK
    gidx = loc + chunk_off[None, None, :]                  # (N, M, 64)
    vals = (bits & np.uint32(0xFFFF0000)).view(np.float32)  # bf16(w) approx

    # top-TSEL candidates per row by device value (33rd kept for the check)
    part = np.argpartition(-vals, TSEL, axis=-1)
    sel = part[..., :TSEL]
    v33 = np.take_along_axis(vals, part[..., TSEL:TSEL + 1], -1)[..., 0]

    cand = np.take_along_axis(gidx, sel, -1)
    ordc = np.argsort(cand, axis=-1)                       # ascending index
    cand = np.take_along_axis(cand, ordc, -1)
    vsel = np.take_along_axis(np.take_along_axis(vals, sel, -1), ordc, -1)

    idx = np.empty((N, M, K), np.int64)
    v16 = np.empty((N, M), np.float32)
    for b in range(N):
        gathered = x64[b][cand[b]]                         # (M, TSEL, D)
        d2 = (
            x2[b][cand[b]] + x2[b][:, None]
            - 2.0 * np.einsum("rd,rcd->rc", x64[b], gathered)
        )
        ordp = np.argsort(d2, axis=-1, kind="stable")[:, :K]
        idx[b] = np.take_along_axis(cand[b], ordp, -1)
        v16[b] = np.take_along_axis(vsel[b], ordp[:, K - 1:K], -1)[:, 0]

    # suspect rows: candidate coverage not provable -> exact recompute
    chunk8 = vals[..., 7::8]                               # (N, M, NCHUNK)
    suspect = (chunk8 >= (v16[..., None] - MARGIN)).any(-1)
    suspect |= v33 >= (v16 - MARGIN)
    LAST_RESULTS["nsuspect"] = int(suspect.sum())
    for b in range(N):
        rows = np.nonzero(suspect[b])[0]
        if rows.size == 0:
            continue
        d2 = (
            x2[b][None, :] + x2[b][rows][:, None]
            - 2.0 * (x64[b][rows] @ x64[b].T)
        )
        part = np.argpartition(d2, 2 * K, axis=-1)[:, :2 * K]
        part = np.sort(part, axis=-1)
        dp = np.take_along_axis(d2, part, -1)
        ordp = np.argsort(dp, axis=-1, kind="stable")[:, :K]
        idx[b, rows] = np.take_along_axis(part, ordp, -1)
    return idx


def kernel(x, k):
    x = np.asarray(x, dtype=np.float32)
    k = int(k)
    assert x.shape == (N, M, D) and k == K

    from concourse.bass_utils import run_bass_kernel_spmd

    if "nc" not in _COMPILED:
        _COMPILED["nc"] = _build_nc(1)
    nc = _COMPILED["nc"]

    in_maps = _prep_inputs(x)
    _r = run_bass_kernel_spmd(nc, in_maps, list(range(NCORES)))
    LAST_RESULTS["exec_time_ns"] = _r.exec_time_ns
    res = _r.results

    bits = np.empty((N, M, NCAND), np.uint32)
    for c in range(NCORES):
        b, h = c // 2, c % 2
        sl = slice(h * QROWS, (h + 1) * QROWS)
        bits[b, sl] = (
            np.ascontiguousarray(res[c]["c8"]).view(np.uint32).reshape(QROWS, NCAND)
        )

    idx = _host_merge(x, bits)
    offset = (np.arange(N, dtype=np.int64) * M)[:, None, None]
    src = (idx + offset).reshape(-1).astype(np.int32)
    dst = np.repeat(np.arange(N * M, dtype=np.int32), K)
    return src, dst


# ---------------------------------------------------------------------------
# benchmarking helpers (not used by the grading path)

def _build_runner(nc):
    """Cached jitted shard_map executor mirroring bass2jax.run_bass_via_pjrt,
    so repeated executions skip retracing and input staging."""
    import jax
    import concourse.mybir as mybir
    from jax.sharding import Mesh, PartitionSpec, NamedSharding
    from jax.experimental.shard_map import shard_map
    from concourse.bass2jax import (
        _bass_exec_p, install_neuronx_cc_hook, partition_id_tensor,
    )

    install_neuronx_cc_hook()
    partition_name = nc.partition_id_tensor.name if nc.partition_id_tensor else None
    in_names, out_names, out_avals, zero_outs = [], [], [], []
    for alloc in nc.m.functions[0].allocations:
        if not isinstance(alloc, mybir.MemoryLocationSet):
            continue
        name = alloc.memorylocations[0].name
        if alloc.kind == "ExternalInput":
            if name != partition_name:
                in_names.append(name)
        elif alloc.kind == "ExternalOutput":
            shape = tuple(alloc.tensor_shape)
            dtype = mybir.dt.np(alloc.dtype)
            out_names.append(name)
            out_avals.append(jax.core.ShapedArray(shape, dtype))
            zero_outs.append(np.zeros(shape, dtype))
    n_params = len(in_names)
    all_names = in_names + out_names
    if partition_name is not None:
        all_names = all_names + [partition_name]

    def _body(*args):
        operands = list(args)
        if partition_name is not None:
            operands.append(partition_id_tensor())
        outs = _bass_exec_p.bind(
            *operands,
            out_avals=tuple(out_avals),
            in_names=tuple(all_names),
            out_names=tuple(out_names),
            lowering_input_output_aliases=(),
            sim_require_finite=True,
            sim_require_nnan=True,
            nc=nc,
        )
        return tuple(outs)

    devices = jax.devices()[:NCORES]
    mesh = Mesh(np.asarray(devices), ("core",))
    nin = n_params + len(out_names)
    fn = jax.jit(
        shard_map(
            _body, mesh=mesh,
            in_specs=(PartitionSpec("core"),) * nin,
            out_specs=(PartitionSpec("core"),) * len(out_names),
            check_rep=False,
        ),
        keep_unused=True,
    )
    sh = NamedSharding(mesh, PartitionSpec("core"))

    def stage(in_maps):
        args = []
        for name in in_names:
            args.append(jax.device_put(
                np.concatenate([np.asarray(m[name]) for m in in_maps], 0), sh))
        for z in zero_outs:
            args.append(jax.device_put(
                np.zeros((NCORES * z.shape[0], *z.shape[1:]), z.dtype), sh))
        return args

    return fn, stage


def benchmark(x, iters=50, reps_hi=8):
    """Estimate per-run device time via run-count scaling."""
    import time

    if "nc" not in _COMPILED:
        _COMPILED["nc"] = _build_nc(1)
    if "nc_hi" not in _COMPILED:
        _COMPILED["nc_hi"] = _build_nc(reps_hi)
    in_maps = _prep_inputs(np.asarray(x, np.float32))

    times = {}
    for key, reps in (("nc", 1), ("nc_hi", reps_hi)):
        fn, stage = _build_runner(_COMPILED[key])
        args = stage(in_maps)
        for _ in range(3):
            out = fn(*args)
            out[0].block_until_ready()
        best = float("inf")
        for _ in range(iters):
            t0 = time.perf_counter()
            out = fn(*args)
            out[0].block_until_ready()
            best = min(best, time.perf_counter() - t0)
        times[reps] = best
    hw_ns = (times[reps_hi] - times[1]) / (reps_hi - 1) * 1e9
    return hw_ns, times


if __name__ == "__main__":
    rng = np.random.default_rng(0)
    xt = rng.standard_normal((N, M, D), dtype=np.float32)
    s, d = kernel(xt, 16)
    print(s[:32], d[:32])


# revision 9
# speedup vs baseline: 1.4747x; 1.4747x over previous
"""KNNGraph (k=16) Bass kernel for 8 NeuronCores.

Input: x (4, 8192, 64) fp32. Output: (src, dst) int32 edge arrays of the
16-NN graph per batch (self included), matching jax.lax.top_k(-d2) order.

Sharding: core c handles batch c//2, key half (c%2)*4096 ... +4096, for
ALL 8192 queries of that batch (key sharding, queries replicated).

Device (per core): the 4096-key slab of w = q.k - |k|^2/2 lives RESIDENT
in PSUM (2 tiles x [128, 2048] fp32 = all 8 banks). Per group of 128
queries:
  PE : 8 K=64 bf16 matmuls (512-wide, alternating array row-tiles
       (0,0)/(64,0) for 2x throughput) accumulate DELTA queries
       (start=False): psum_g = bias + sum_{j<=g} dq_j . k, where
       dq_g = bf16(q_g - effective_prev). The -|k|^2/2 bias is seeded
       once per rep by rank-1 ones x (bias_hi; bias_lo) matmuls, all on
       the PE (in-order; avoids cross-engine WAW races on PSUM).
  ACT: per 2048-half, one fp32->bf16 convert into the HIGH u16 halves
       of a u32 "packed" tile (low halves = constant u16 iota), i.e.
       packed = bf16(w)<<16 | key_index.
  DVE: per 1024-quarter, one max8 over packed-viewed-as-fp32 -> top-8
       (value, index) pairs in a single pass; 32 candidates per row.
Host: 64 candidates/row from the core pair, exact fp64 re-rank of the
top-32 by device value, suspect detection (chunk-8th or 33rd candidate
near the 16th pick) with exact recompute for the rare flagged rows.
"""

import numpy as np

N, M, D = 4, 8192, 64
K = 16
NCORES = 8
KH = M // 2              # keys per core
NG = M // 128            # 64 query groups per core
HALF = 2048              # keys per ACT convert / psum tile
NCAND = 32               # candidates per row per core (4 quarters x 8)
TSEL = 32                # host: exact-rank this many of the 64 merged
MARGIN = 1.0             # suspect margin in device w units

_COMPILED = {}
LAST_RESULTS = {}


def _build_nc(reps=1):
    import concourse.bacc as bacc
    import concourse.mybir as mybir
    import concourse.tile as tile

    nc = bacc.Bacc(None)
    f32 = mybir.dt.float32
    bf16 = mybir.dt.bfloat16
    u32 = mybir.dt.uint32
    Copy = mybir.ActivationFunctionType.Copy

    q_d = nc.declare_dram_parameter("q", [D, M], bf16, isOutput=False)
    kv_d = nc.declare_dram_parameter("kv", [D, KH], bf16, isOutput=False)
    bias_d = nc.declare_dram_parameter("bias", [2, KH], bf16, isOutput=False)
    ones_d = nc.declare_dram_parameter("ones", [2, 128], bf16, isOutput=False)
    pinit_d = nc.declare_dram_parameter("pinit", [128, 2 * HALF], u32, isOutput=False)
    c8_d = nc.declare_dram_parameter("c8", [NG, 128, NCAND], f32, isOutput=True)

    with tile.TileContext(nc) as tc:
        with (
            tc.tile_pool(name="singles", bufs=1) as singles,
            tc.tile_pool(name="psum", bufs=1, space="PSUM") as psum,
            tc.tile_pool(name="cands", bufs=2) as cands,
        ):
            # query deltas, split into 4 tiles (access-count caps), rows
            # 0-63 and 64-127 hold the same data (for the two PE row-tiles)
            q_sb = [singles.tile([128, 2048], bf16, name=f"qs{i}") for i in range(4)]
            # key chunks, split into 4 tiles of 1024
            kv_sb = [singles.tile([128, 1024], bf16, name=f"kvs{i}") for i in range(4)]
            bias_sb = singles.tile([2, KH], bf16)
            ones_sb = singles.tile([2, 128], bf16)
            packedA = singles.tile([128, HALF], u32)
            packedB = singles.tile([128, HALF], u32)
            for i in range(4):
                nc.gpsimd.dma_start(out=q_sb[i][0:D], in_=q_d[:, i * 2048:(i + 1) * 2048])
                nc.gpsimd.dma_start(out=q_sb[i][64:128], in_=q_d[:, i * 2048:(i + 1) * 2048])
                nc.gpsimd.dma_start(out=kv_sb[i][0:D], in_=kv_d[:, i * 1024:(i + 1) * 1024])
                nc.gpsimd.dma_start(out=kv_sb[i][64:128], in_=kv_d[:, i * 1024:(i + 1) * 1024])
            nc.gpsimd.dma_start(out=bias_sb[:], in_=bias_d[:])
            nc.gpsimd.dma_start(out=ones_sb[:], in_=ones_d[:])
            nc.gpsimd.dma_start(out=packedA[:], in_=pinit_d[:, 0:HALF])
            nc.gpsimd.dma_start(out=packedB[:], in_=pinit_d[:, HALF:2 * HALF])

            pA = psum.tile([128, HALF], f32, name="pA")
            pB = psum.tile([128, HALF], f32, name="pB")

            def hi_view(packed):
                return (packed[:].bitcast(bf16)
                        .rearrange("p (n two) -> p n two", two=2)[:, :, 1])

            for _ in range(reps):
                # seed psum with -|k|^2/2 (hi+lo rows, ones weights) on PE
                for t, p in ((0, pA), (1, pB)):
                    for s in range(4):
                        nc.tensor.matmul(
                            p[:, s * 512:(s + 1) * 512], ones_sb[:],
                            bias_sb[:, t * HALF + s * 512:t * HALF + (s + 1) * 512],
                            start=True, stop=True)
                mmi = 0
                for g in range(NG):
                    c8 = cands.tile([128, NCAND], f32, tag="c8")
                    qt = q_sb[g // 16]
                    qoff = (g % 16) * 128
                    for half, (p, packed) in enumerate(((pA, packedA), (pB, packedB))):
                        for s in range(4):
                            base = 64 * (mmi % 2)
                            mmi += 1
                            kt = kv_sb[half * 2 + s // 2]
                            koff = (s % 2) * 512
                            nc.tensor.matmul(
                                p[:, s * 512:(s + 1) * 512],
                                qt[base:base + D, qoff:qoff + 128],
                                kt[base:base + D, koff:koff + 512],
                                start=False, stop=True, skip_group_check=True)
                        nc.scalar.activation(out=hi_view(packed), in_=p[:], func=Copy)
                        nc.vector.max(
                            out=c8[:, half * 16:half * 16 + 8],
                            in_=packed[:, 0:1024].bitcast(f32))
                        nc.vector.max(
                            out=c8[:, half * 16 + 8:half * 16 + 16],
                            in_=packed[:, 1024:2048].bitcast(f32))
                    nc.sync.dma_start(out=c8_d[g], in_=c8[:])
    if not nc.is_finalized():
        nc.finalize()
    return nc


def _prep_inputs(x):
    """Per-core input dicts. x: (N, M, D) fp32."""
    import ml_dtypes

    bf = ml_dtypes.bfloat16
    x64 = x.astype(np.float64)
    x2 = (x64 * x64).sum(-1)                  # (N, M)
    nh = (-0.5 * x2).astype(np.float32)
    nh_hi = nh.astype(bf)
    nh_lo = (nh - nh_hi.astype(np.float32)).astype(bf)

    iota = np.arange(HALF, dtype=np.uint32)
    pinit = np.tile(np.concatenate([iota, iota])[None, :], (128, 1)).copy()
    ones = np.ones((2, 128), bf)

    # delta-encoded query columns per batch: (D, M) bf16
    qdelta = []
    for b in range(N):
        cols = np.empty((D, M), bf)
        S = np.zeros((D, 128), np.float32)
        for g in range(NG):
            Q = x[b, g * 128:(g + 1) * 128, :].T        # (D, 128) fp32
            Dg = (Q - S).astype(bf)
            S += Dg.astype(np.float32)
            cols[:, g * 128:(g + 1) * 128] = Dg
        qdelta.append(cols)

    in_maps = []
    for c in range(NCORES):
        b, h = c // 2, c % 2
        sl = slice(h * KH, (h + 1) * KH)
        kv = x[b, sl, :].T.astype(bf)                   # (D, KH)
        bias = np.stack([nh_hi[b, sl], nh_lo[b, sl]])   # (2, KH)
        in_maps.append({
            "q": qdelta[b], "kv": np.ascontiguousarray(kv),
            "bias": np.ascontiguousarray(bias), "ones": ones, "pinit": pinit,
        })
    return in_maps


def _host_merge(x, bits):
    """bits: (N, M, 64) u32 packed candidates (both key halves).
    Returns idx (N, M, K) i64."""
    x64 = x.astype(np.float64)
    x2 = (x64 * x64).sum(-1)                              # (N, M)
    loc = (bits & np.uint32(0xFFFF)).astype(np.int64)
    # slot s of 64: s//32 = key half, (s%32)//16 = 2048-half, each half's
    # iota spans 0..2047
    half_off = (np.arange(64, dtype=np.int64) // 32) * KH
    quart_off = ((np.arange(64, dtype=np.int64) % 32) // 16) * HALF
    gidx = loc + (half_off + quart_off)[None, None, :]     # (N, M, 64)
    vals = (bits & np.uint32(0xFFFF0000)).view(np.float32)

    part = np.argpartition(-vals, TSEL, axis=-1)
    sel = part[..., :TSEL]
    v33 = np.take_along_axis(vals, part[..., TSEL:TSEL + 1], -1)[..., 0]

    cand = np.take_along_axis(gidx, sel, -1)
    ordc = np.argsort(cand, axis=-1)
    cand = np.take_along_axis(cand, ordc, -1)
    vsel = np.take_along_axis(np.take_along_axis(vals, sel, -1), ordc, -1)

    idx = np.empty((N, M, K), np.int64)
    v16 = np.empty((N, M), np.float32)
    for b in range(N):
        gathered = x64[b][cand[b]]                         # (M, TSEL, D)
        d2 = (
            x2[b][cand[b]] + x2[b][:, None]
            - 2.0 * np.einsum("rd,rcd->rc", x64[b], gathered)
        )
        ordp = np.argsort(d2, axis=-1, kind="stable")[:, :K]
        idx[b] = np.take_along_axis(cand[b], ordp, -1)
        v16[b] = np.take_along_axis(vsel[b], ordp[:, K - 1:K], -1)[:, 0]

    # suspect rows: candidate coverage not provable -> exact recompute
    chunk8 = vals[..., 7::8]                               # 8 chunk-of-1024 8ths
    suspect = (chunk8 >= (v16[..., None] - MARGIN)).any(-1)
    suspect |= v33 >= (v16 - MARGIN)
    LAST_RESULTS["nsuspect"] = int(suspect.sum())
    for b in range(N):
        rows = np.nonzero(suspect[b])[0]
        if rows.size == 0:
            continue
        d2 = (
            x2[b][None, :] + x2[b][rows][:, None]
            - 2.0 * (x64[b][rows] @ x64[b].T)
        )
        part2 = np.argpartition(d2, 2 * K, axis=-1)[:, :2 * K]
        part2 = np.sort(part2, axis=-1)
        dp = np.take_along_axis(d2, part2, -1)
        ordp = np.argsort(dp, axis=-1, kind="stable")[:, :K]
        idx[b, rows] = np.take_along_axis(part2, ordp, -1)
    return idx


def kernel(x, k):
    x = np.asarray(x, dtype=np.float32)
    k = int(k)
    assert x.shape == (N, M, D) and k == K

    from concourse.bass_utils import run_bass_kernel_spmd

    if "nc" not in _COMPILED:
        _COMPILED["nc"] = _build_nc(1)
    nc = _COMPILED["nc"]

    in_maps = _prep_inputs(x)
    _r = run_bass_kernel_spmd(nc, in_maps, list(range(NCORES)))
    LAST_RESULTS["exec_time_ns"] = _r.exec_time_ns
    res = _r.results

    bits = np.empty((N, M, 2 * NCAND), np.uint32)
    for c in range(NCORES):
        b, h = c // 2, c % 2
        sl = slice(h * NCAND, (h + 1) * NCAND)
        bits[b, :, sl] = (
            np.ascontiguousarray(res[c]["c8"]).view(np.uint32).reshape(M, NCAND)
        )

    idx = _host_merge(x, bits)
    offset = (np.arange(N, dtype=np.int64) * M)[:, None, None]
    src = (idx + offset).reshape(-1).astype(np.int32)
    dst = np.repeat(np.arange(N * M, dtype=np.int32), K)
    return src, dst


# ---------------------------------------------------------------------------
# benchmarking helpers (not used by the grading path)

def _build_runner(nc):
    """Cached jitted shard_map executor mirroring bass2jax.run_bass_via_pjrt,
    so repeated executions skip retracing and input staging."""
    import jax
    import concourse.mybir as mybir
    from jax.sharding import Mesh, PartitionSpec, NamedSharding
    from jax.experimental.shard_map import shard_map
    from concourse.bass2jax import (
        _bass_exec_p, install_neuronx_cc_hook, partition_id_tensor,
    )

    install_neuronx_cc_hook()
    partition_name = nc.partition_id_tensor.name if nc.partition_id_tensor else None
    in_names, out_names, out_avals, zero_outs = [], [], [], []
    for alloc in nc.m.functions[0].allocations:
        if not isinstance(alloc, mybir.MemoryLocationSet):
            continue
        name = alloc.memorylocations[0].name
        if alloc.kind == "ExternalInput":
            if name != partition_name:
                in_names.append(name)
        elif alloc.kind == "ExternalOutput":
            shape = tuple(alloc.tensor_shape)
            dtype = mybir.dt.np(alloc.dtype)
            out_names.append(name)
            out_avals.append(jax.core.ShapedArray(shape, dtype))
            zero_outs.append(np.zeros(shape, dtype))
    n_params = len(in_names)
    all_names = in_names + out_names
    if partition_name is not None:
        all_names = all_names + [partition_name]

    def _body(*args):
        operands = list(args)
        if partition_name is not None:
            operands.append(partition_id_tensor())
        outs = _bass_exec_p.bind(
            *operands,
            out_avals=tuple(out_avals),
            in_names=tuple(all_names),
            out_names=tuple(out_names),
            lowering_input_output_aliases=(),
            sim_require_finite=True,
            sim_require_nnan=True,
            nc=nc,
        )
        return tuple(outs)

    devices = jax.devices()[:NCORES]
    mesh = Mesh(np.asarray(devices), ("core",))
    nin = n_params + len(out_names)
    fn = jax.jit(
        shard_map(
            _body, mesh=mesh,
            in_specs=(PartitionSpec("core"),) * nin,
            out_specs=(PartitionSpec("core"),) * len(out_names),
            check_rep=False,
        ),
        keep_unused=True,
    )
    sh = NamedSharding(mesh, PartitionSpec("core"))

    def stage(in_maps):
        args = []
        for name in in_names:
            args.append(jax.device_put(
                np.concatenate([np.asarray(m[name]) for m in in_maps], 0), sh))
        for z in zero_outs:
            args.append(jax.device_put(
                np.zeros((NCORES * z.shape[0], *z.shape[1:]), z.dtype), sh))
        return args

    return fn, stage


def benchmark(x, iters=50, reps_hi=16):
    """Estimate per-run device time via run-count scaling."""
    import time

    if "nc" not in _COMPILED:
        _COMPILED["nc"] = _build_nc(1)
    if "nc_hi" not in _COMPILED:
        _COMPILED["nc_hi"] = _build_nc(reps_hi)
    in_maps = _prep_inputs(np.asarray(x, np.float32))

    runners = {}
    for key, reps in (("nc", 1), ("nc_hi", reps_hi)):
        fn, stage = _build_runner(_COMPILED[key])
        args = stage(in_maps)
        for _ in range(3):
            out = fn(*args)
            out[0].block_until_ready()
        runners[reps] = (fn, args)

    # interleave measurements so slow drift in the fixed per-call overhead
    # cancels in the pairwise difference
    diffs = []
    times = {1: float("inf"), reps_hi: float("inf")}
    for _ in range(iters):
        got = {}
        for reps in (1, reps_hi):
            fn, args = runners[reps]
            t0 = time.perf_counter()
            out = fn(*args)
            np.asarray(out[0])  # D2H forces true device completion
            got[reps] = time.perf_counter() - t0
            times[reps] = min(times[reps], got[reps])
        diffs.append(got[reps_hi] - got[1])
    diffs.sort()
    med = diffs[len(diffs) // 2]
    hw_ns = med / (reps_hi - 1) * 1e9
    return hw_ns, times


if __name__ == "__main__":
    rng = np.random.default_rng(0)
    xt = rng.standard_normal((N, M, D), dtype=np.float32)
    s, d = kernel(xt, 16)
    print(s[:32], d[:32])
